# revision 7
# baseline (speedup 1.0000x reference)
"""Block-sparse attention Trainium2 kernel (v2).

Problem: nn_BlockSparseAttention (B=4, N=8256=64x129 tokens, D=1024,
H=8 heads, DK=DV=64, BLK=129). Full computation:
  q,k,v = x@Wq, x@Wk, x@Wv (per-head reshape)
  block-local softmax attention within each 129-token block
  global attention: slot-0 token of each block attends over all blocks'
  slot-0 tokens; its output is *added* to the local output at slot 0
  y = out @ Wo + bo

Sharding: 64 blocks split 8 ways (8 contiguous blocks per core, all 4
batches). Global-token K/V (64 tokens/batch) are computed redundantly on
every core from a transposed xgt input (the slot-0 rows of x), so no
collectives are needed.

v2 changes vs v1:
  - weights are baked into the NEFF as constants (bf16, already in the
    SBUF layouts the matmuls want) -- loaded to HBM once at model load,
    not streamed per inference.
  - activations are shipped pre-transposed (bf16 x^T per core), so the
    on-device PE transpose stage and its ACT copies are gone.
  - the output is produced transposed (yT, fp16) and un-permuted on the
    host; out-projection keeps tokens on the free dim so the 8-token
    tail slice is cheap.
  - all I/O is 16-bit: per-core per-inference HBM+host traffic drops
    from ~43 MB to ~17.4 MB.
  - global slot-0 add is one strided DVE op per batch instead of 32.
  - PSUM->SBUF copies balanced between ACT and DVE.

On-device pipeline (all matmuls bf16 inputs, fp32 PSUM accumulation):
  - qT/kT = W^T @ xT stay feature-on-partition; v = x@Wv
    token-on-partition (from xT with x-chunk as the stationary operand).
  - scores are computed transposed, sT[j, i] = k_j . q_i, so the
    attention-weights matmul (PV) needs no transposes; softmax
    denominators come from a ones-vector matmul; exp runs on the scalar
    engine reading PSUM directly (scale=1/sqrt(DK) folded in). Scores
    are O(1) so the max-subtraction is skipped (exp is safe in fp32).
  - normalization multiplies the PV output by a broadcast reciprocal
    (broadcast across partitions via a tiny selector matmul).
  - yT = Wo^T @ outT + bo, bias added during the PSUM->SBUF copy.
"""

import numpy as np

H, BLK, DK, DV = 8, 129, 64, 64
B, N, D = 4, 8256, 1024
INNER = H * DK           # 512
NB = N // BLK            # 64 blocks
NCORES = 8
NBC = NB // NCORES       # 8 blocks per core
T = NBC * BLK            # 1032 tokens per core per batch
DC = D // 128            # 8 contraction chunks over D
FC = INNER // 128        # 4 chunks over the 512 inner dim
TSL = [(0, 512), (512, 512), (1024, T - 1024)]

_NC_CACHE = {}


def _weight_layouts(Wq, Wk, Wv, Wo, bo):
    """Precompute the SBUF-layout weight constants (bf16/fp32 numpy)."""
    import ml_dtypes
    bf16 = ml_dtypes.bfloat16
    Wq = np.asarray(Wq, np.float32)
    Wk = np.asarray(Wk, np.float32)
    Wv = np.asarray(Wv, np.float32)
    Wo = np.asarray(Wo, np.float32)
    bo = np.asarray(bo, np.float32)

    # Wq/Wk interleaved so head h lives at (chunk h%4, partition 64*(h//4)):
    # w_sb[p, c, m*128 + 64*a + d] = W[c*128 + p, a*256 + m*64 + d]
    def qk_layout(w):
        w5 = w.reshape(DC, 128, 2, 4, 64)          # [c, p, a, m, d]
        return np.ascontiguousarray(
            w5.transpose(1, 0, 3, 2, 4).reshape(128, DC, INNER)).astype(bf16)

    wq_sb = qk_layout(Wq)
    wk_sb = qk_layout(Wk)
    # Wv standard: w_sb[p, c, f] = Wv[c*128 + p, f]
    wv_sb = np.ascontiguousarray(
        Wv.reshape(DC, 128, INNER).transpose(1, 0, 2)).astype(bf16)
    # Wo: w_sb[p, c, f] = Wo[c*128 + p, f]   (c over INNER chunks)
    wo_sb = np.ascontiguousarray(
        Wo.reshape(FC, 128, D).transpose(1, 0, 2)).astype(bf16)
    # bias column per output-feature chunk: bo_col[p, fch] = bo[fch*128+p]
    bo_col = np.ascontiguousarray(bo.reshape(8, 128).T).astype(np.float32)
    # partition-broadcast selector: E2[0, 0:64]=1, E2[1, 64:128]=1
    e2 = np.zeros((2, 128), dtype=bf16)
    e2[0, 0:64] = 1.0
    e2[1, 64:128] = 1.0
    return wq_sb, wk_sb, wv_sb, wo_sb, bo_col, e2


def _build_nc(Wq, Wk, Wv, Wo, bo):
    import concourse.bacc as bacc
    import concourse.tile as tile
    from concourse import mybir

    f32 = mybir.dt.float32
    bf16 = mybir.dt.bfloat16
    f16 = mybir.dt.float16

    wq_np, wk_np, wv_np, wo_np, bo_np, e2_np = _weight_layouts(
        Wq, Wk, Wv, Wo, bo)

    nc = bacc.Bacc("TRN2", target_bir_lowering=False, debug=False,
                   num_devices=NCORES)

    # I/O (16-bit, pre-transposed on host)
    xt = nc.dram_tensor("xt", [B, 128, DC, T], bf16, kind="ExternalInput").ap()
    xgt = nc.dram_tensor("xgt", [B, 128, DC, NB], bf16,
                         kind="ExternalInput").ap()
    y = nc.dram_tensor("y", [B, 128, DC, T], f16, kind="ExternalOutput").ap()

    # weights inlined into the NEFF
    wq_d = nc.inline_tensor(wq_np, name="wq_c").ap()
    wk_d = nc.inline_tensor(wk_np, name="wk_c").ap()
    wv_d = nc.inline_tensor(wv_np, name="wv_c").ap()
    wo_d = nc.inline_tensor(wo_np, name="wo_c").ap()
    bo_d = nc.inline_tensor(bo_np, name="bo_c").ap()
    e2_d = nc.inline_tensor(e2_np, name="e2_c").ap()

    ex = mybir.ActivationFunctionType.Exp
    idf = mybir.ActivationFunctionType.Identity

    with tile.TileContext(nc) as tc:
        with (
            tc.tile_pool(name="const", bufs=1) as const,
            tc.tile_pool(name="batch", bufs=2) as bp,
            tc.tile_pool(name="single", bufs=1) as sg1,
            tc.tile_pool(name="att", bufs=3) as ap_,
            tc.tile_pool(name="ppsum", bufs=8, space="PSUM") as pp,
        ):
            stp = ogp = smp = pp
            # ---- constants ----
            ones_col = const.tile([128, 1], bf16)
            nc.vector.memset(ones_col, 1.0)
            ones_row = const.tile([1, 128], bf16)
            nc.vector.memset(ones_row, 1.0)
            e2 = const.tile([2, 128], bf16)
            nc.sync.dma_start(out=e2, in_=e2_d)
            wq_sb = const.tile([128, DC, INNER], bf16)
            wk_sb = const.tile([128, DC, INNER], bf16)
            wv_sb = const.tile([128, DC, INNER], bf16)
            wo_sb = const.tile([128, FC, D], bf16)
            bo_col = const.tile([128, DC], f32)
            nc.sync.dma_start(out=wq_sb, in_=wq_d)
            nc.sync.dma_start(out=wk_sb, in_=wk_d)
            nc.sync.dma_start(out=wv_sb, in_=wv_d)
            nc.sync.dma_start(out=wo_sb, in_=wo_d)
            nc.sync.dma_start(out=bo_col, in_=bo_d)
            # global tokens x^T for all batches, loaded once
            xgT = sg1.tile([128, B, DC, NB], bf16, tag="xgT")
            nc.sync.dma_start(
                out=xgT, in_=xgt.rearrange("b p c n -> p b c n"))

            for b in range(B):
                # ---- load x^T for this batch ----
                xT = bp.tile([128, DC, T], bf16, tag="xT")
                nc.sync.dma_start(out=xT, in_=xt[b])

                # ---- global tokens: kgT, vg ----
                kgT = bp.tile([128, FC, NB], bf16, tag="kgT")
                for mc in range(FC):
                    pt = smp.tile([128, NB], f32, tag="pp")
                    for dc in range(DC):
                        nc.tensor.matmul(
                            pt, wk_sb[:, dc, mc * 128:(mc + 1) * 128],
                            xgT[:, b, dc, :],
                            start=(dc == 0), stop=(dc == DC - 1))
                    nc.vector.tensor_copy(out=kgT[:, mc, :], in_=pt)
                vg = bp.tile([64, INNER], bf16, tag="vg")
                pt = pp.tile([128, 512], f32, tag="pp")
                for dc in range(DC):
                    nc.tensor.matmul(pt[:64, :], xgT[:, b, dc, 0:64],
                                     wv_sb[:, dc, :],
                                     start=(dc == 0), stop=(dc == DC - 1))
                nc.vector.tensor_copy(out=vg, in_=pt[:64, :])

                # ---- q/k projections (transposed layout) ----
                qT = bp.tile([128, FC, T], bf16, tag="qT")
                kT = bp.tile([128, FC, T], bf16, tag="kT")
                for dst, w_sb, eng in ((qT, wq_sb, "act"), (kT, wk_sb, "dve")):
                    for mc in range(FC):
                        for t0, tsz in TSL:
                            pt = pp.tile([128, 512], f32, tag="pp")
                            for dc in range(DC):
                                nc.tensor.matmul(
                                    pt[:, :tsz],
                                    w_sb[:, dc, mc * 128:(mc + 1) * 128],
                                    xT[:, dc, t0:t0 + tsz],
                                    start=(dc == 0), stop=(dc == DC - 1))
                            if eng == "act":
                                nc.scalar.copy(
                                    out=dst[:, mc, t0:t0 + tsz],
                                    in_=pt[:, :tsz])
                            else:
                                nc.vector.tensor_copy(
                                    out=dst[:, mc, t0:t0 + tsz],
                                    in_=pt[:, :tsz])

                # ---- v projection (token-on-partition, per block) ----
                v = bp.tile([128, NBC, INNER], bf16, tag="v")
                for n in range(NBC):
                    pt = pp.tile([128, 512], f32, tag="pp")
                    for dc in range(DC):
                        nc.tensor.matmul(
                            pt, xT[:, dc, n * BLK:n * BLK + 128],
                            wv_sb[:, dc, :],
                            start=(dc == 0), stop=(dc == DC - 1))
                    nc.vector.tensor_copy(out=v[:, n, :], in_=pt)
                # last token of each block, batched: tokens 129n+128
                vl8 = bp.tile([NBC, INNER], bf16, tag="vl8")
                pt = pp.tile([128, 512], f32, tag="pp")
                for dc in range(DC):
                    nc.tensor.matmul(pt[:NBC, :], xT[:, dc, 128::BLK],
                                     wv_sb[:, dc, :],
                                     start=(dc == 0), stop=(dc == DC - 1))
                nc.vector.tensor_copy(out=vl8, in_=pt[:NBC, :])
                vl_all = bp.tile([1, NBC, INNER], bf16, tag="vlall")
                nc.sync.dma_start(out=vl_all, in_=vl8)

                outT = bp.tile([128, FC, T], bf16, tag="outT")

                # ---- global attention for this core's 8 blocks ----
                eg = bp.tile([64, H, NB // NCORES], bf16, tag="eg")
                lg = smp.tile([1, H * NBC], f32, tag="pp")
                for h in range(H):
                    p0 = 64 * (h // 4)
                    hc = h % 4
                    sgt = smp.tile([64, NBC], f32, tag="pp")
                    nc.tensor.matmul(sgt, kgT[p0:p0 + 64, hc, :],
                                     qT[p0:p0 + 64, hc, 0::BLK],
                                     start=True, stop=True)
                    nc.scalar.activation(
                        out=eg[:, h, :], in_=sgt, func=ex, scale=0.125)
                    nc.tensor.matmul(lg[:, h * NBC:(h + 1) * NBC],
                                     ones_col[0:64, :], eg[:, h, :],
                                     start=True, stop=True)
                rlg = bp.tile([1, H * NBC], bf16, tag="rlg")
                with nc.allow_low_precision("1/l to bf16"):
                    nc.vector.reciprocal(out=rlg, in_=lg)
                ogn = bp.tile([128, FC, NBC], bf16, tag="ogn")
                for hp in range(4):
                    ogg = smp.tile([128, NBC], f32, tag="pp")
                    for hh in range(2):
                        h = 2 * hp + hh
                        nc.tensor.matmul(
                            ogg[64 * hh:64 * hh + 64, :],
                            vg[:, h * DV:(h + 1) * DV], eg[:, h, :],
                            start=True, stop=True)
                    rlbg = smp.tile([128, NBC], f32, tag="pp")
                    for hh in range(2):
                        o0 = hp * 2 * NBC + hh * NBC
                        nc.tensor.matmul(
                            rlbg[64 * hh:64 * hh + 64, :],
                            ones_row[0:1, 0:64],
                            rlg[0:1, o0:o0 + NBC],
                            start=True, stop=True)
                    rlbg_sb = bp.tile([128, NBC], bf16, tag="rlbg_sb")
                    nc.scalar.copy(out=rlbg_sb, in_=rlbg)
                    nc.vector.tensor_mul(out=ogn[:, hp, :], in0=ogg,
                                         in1=rlbg_sb)

                # ---- block-local attention ----
                for n in range(NBC):
                    c0 = n * BLK
                    eT = ap_.tile([128, H, BLK], bf16, tag="eT")
                    eTl = ap_.tile([1, H, BLK], bf16, tag="eTl")
                    rl = ap_.tile([1, H * BLK], bf16, tag="rl")
                    for hp in range(4):
                        st = stp.tile([128, 2 * BLK], f32, tag="pp")
                        stl = smp.tile([1, 2 * BLK], f32, tag="pp")
                        for hh in range(2):
                            h = 2 * hp + hh
                            p0 = 64 * (h // 4)
                            hc = h % 4
                            lq = qT[p0:p0 + 64, hc, c0:c0 + BLK]
                            nc.tensor.matmul(
                                st[:, hh * BLK:(hh + 1) * BLK],
                                kT[p0:p0 + 64, hc, c0:c0 + 128], lq,
                                start=True, stop=True)
                            nc.tensor.matmul(
                                stl[:, hh * BLK:(hh + 1) * BLK],
                                kT[p0:p0 + 64, hc, c0 + 128:c0 + BLK], lq,
                                start=True, stop=True)
                        nc.scalar.activation(
                            out=eT[:, 2 * hp:2 * hp + 2, :], in_=st,
                            func=ex, scale=0.125)
                        nc.scalar.activation(
                            out=eTl[:, 2 * hp:2 * hp + 2, :], in_=stl,
                            func=ex, scale=0.125)
                        lp = smp.tile([1, 2 * BLK], f32, tag="pp")
                        nc.tensor.matmul(lp, ones_col,
                                         eT[:, 2 * hp:2 * hp + 2, :],
                                         start=True, stop=False)
                        nc.tensor.matmul(lp, ones_col[0:1, :],
                                         eTl[:, 2 * hp:2 * hp + 2, :],
                                         start=False, stop=True)
                        with nc.allow_low_precision("1/l to bf16"):
                            nc.vector.reciprocal(
                                out=rl[:, hp * 2 * BLK:(hp + 1) * 2 * BLK],
                                in_=lp)
                    for hp in range(4):
                        og = ogp.tile([128, BLK], f32, tag="pp")
                        for hh in range(2):
                            h = 2 * hp + hh
                            nc.tensor.matmul(
                                og[64 * hh:64 * hh + 64, :],
                                v[:, n, h * DV:(h + 1) * DV],
                                eT[:, h, :], start=True, stop=False)
                            nc.tensor.matmul(
                                og[64 * hh:64 * hh + 64, :],
                                vl_all[0:1, n, h * DV:(h + 1) * DV],
                                eTl[:, h, :], start=False, stop=True)
                        rlb = ogp.tile([128, BLK], f32, tag="pp")
                        for hh in range(2):
                            o0 = hp * 2 * BLK + hh * BLK
                            nc.tensor.matmul(
                                rlb[64 * hh:64 * hh + 64, :],
                                ones_row[0:1, 0:64],
                                rl[0:1, o0:o0 + BLK],
                                start=True, stop=True)
                        rlb_sb = ap_.tile([128, BLK], bf16, tag="rlb_sb")
                        nc.vector.tensor_copy(out=rlb_sb, in_=rlb)
                        nc.vector.tensor_mul(
                            out=outT[:, hp, c0:c0 + BLK], in0=og,
                            in1=rlb_sb)

                # global slot-0 add for all 8 blocks at once (strided)
                nc.vector.tensor_add(
                    out=outT[:, :, 0::BLK], in0=outT[:, :, 0::BLK], in1=ogn)

                # ---- output projection + bias (tokens on free dim) ----
                yT = bp.tile([128, DC, T], f16, tag="yT")
                for fch in range(DC):
                    for i, (t0, tsz) in enumerate(TSL):
                        yp = pp.tile([128, 512], f32, tag="pp")
                        for fc in range(FC):
                            nc.tensor.matmul(
                                yp[:, :tsz],
                                wo_sb[:, fc, fch * 128:(fch + 1) * 128],
                                outT[:, fc, t0:t0 + tsz],
                                start=(fc == 0), stop=(fc == FC - 1))
                        if (fch + i) % 2 == 0:
                            nc.scalar.activation(
                                out=yT[:, fch, t0:t0 + tsz],
                                in_=yp[:, :tsz], func=idf,
                                bias=bo_col[:, fch:fch + 1])
                        else:
                            nc.vector.tensor_scalar_add(
                                out=yT[:, fch, t0:t0 + tsz],
                                in0=yp[:, :tsz],
                                scalar1=bo_col[:, fch:fch + 1])
                nc.sync.dma_start(out=y[b], in_=yT)

    nc.compile()
    return nc


def _key(*arrs):
    import hashlib
    m = hashlib.sha1()
    for a in arrs:
        m.update(np.ascontiguousarray(a, dtype=np.float32).tobytes())
    return m.hexdigest()


def _get_nc(Wq, Wk, Wv, Wo, bo):
    k = _key(Wq, Wk, Wv, Wo, bo)
    if k not in _NC_CACHE:
        _NC_CACHE[k] = _build_nc(Wq, Wk, Wv, Wo, bo)
    return _NC_CACHE[k]


def prep_core_inputs(x):
    """Host-side layout prep: per-core transposed bf16 activations."""
    import ml_dtypes
    bf16 = ml_dtypes.bfloat16
    x = np.asarray(x, dtype=np.float32)
    xg = x[:, ::BLK, :]                            # [B, NB, D]
    xgt = np.ascontiguousarray(
        xg.reshape(B, NB, DC, 128).transpose(0, 3, 2, 1)).astype(bf16)
    in_maps = []
    for c in range(NCORES):
        xs = x[:, c * T:(c + 1) * T, :]            # [B, T, D]
        xtc = np.ascontiguousarray(
            xs.reshape(B, T, DC, 128).transpose(0, 3, 2, 1)).astype(bf16)
        in_maps.append({"xt": xtc, "xgt": xgt})
    return in_maps


def unpack_output(res_list):
    """[NCORES] of y [B, 128, DC, T] fp16 -> full [B, N, D] fp32."""
    parts = []
    for c in range(NCORES):
        yt = np.asarray(res_list[c]).astype(np.float32)   # [B,128,DC,T]
        parts.append(yt.transpose(0, 3, 2, 1).reshape(B, T, D))
    return np.concatenate(parts, axis=1)


def kernel(x, Wq, Wk, Wv, Wo, bo):
    from concourse.bass_utils import run_bass_kernel_spmd

    nc = _get_nc(Wq, Wk, Wv, Wo, bo)
    in_maps = prep_core_inputs(x)
    res = run_bass_kernel_spmd(nc, in_maps, core_ids=list(range(NCORES)))
    return unpack_output([res.results[c]["y"] for c in range(NCORES)])
